# revision 20
# baseline (speedup 1.0000x reference)
"""AdaptiveKernelFC Trainium2 kernel (8-core data parallel).

Math: the reference builds per-sample filters w[n,p,c,kh,kw] =
x[n,c,kh,kw]*Wk[p] + bk[p] and convolves x[n] with them (7x7 kernel ==
feature map size, pad 3).  The conv factors exactly:

    y[n,p,i,j] = Wk[p]*S1[n,i,j] + bk[p]*S2[n,i,j] + b_adap[p]

with
    S1[n,i,j] = sum_{a,b} G[n,(a,b),(a+i-3,b+j-3)]      (Gram diag bands)
    G[n,r,q]  = sum_c x[n,c,r] * x[n,c,q]               (49x49 per sample)
    S2[n,i,j] = sum_{a,b} xspad[n,i+a,j+b],  xs = sum_c x[n,c]

Pipeline per core (4 samples):
  1. One fused matmul pair per sample: lhsT = [x_chunk | ones49] so PSUM
     rows 0-48 are the Gram matrix and rows 49-97 are 49 replicated
     copies of the channel sums.
  2. DVE-copy into a zero-padded 13x13 layout (98, B, 13, 13).
  3. Dump to DRAM with a per-row stagger: row r=(a,b) lands at
     683*r - 13a - b.  This makes the diagonal-band gather uniform:
     element (r, n, i*13+j') sits at 683*r + 169*n + (i*13+j'), so ONE
     3-dim DMA gathers every 91-wide band for all 98 rows x 4 samples.
  4. One selector matmul (98,2)^T @ (98, B,7,7 view) -> [S1; S2].
  5. K=2 matmul against [Wk; bk], then ScalarE Identity-with-bias adds
     b_adap while moving PSUM -> SBUF.

Sharding: pure data parallel, batch N=32 split 4 samples/core across 8
cores; params replicated; outputs concatenated.
"""

import os
import numpy as np

import concourse.bass as bass
import concourse.bacc as bacc
import concourse.mybir as mybir
import concourse.tile as tile
import ml_dtypes

jnp_bf16 = ml_dtypes.bfloat16
from concourse.ap import AP
from concourse.bass_utils import run_bass_kernel_spmd

N, C, H, W = 32, 256, 7, 7
P = 256
NCORES = 8
B = N // NCORES          # samples per core
HW = H * W               # 49
PW = 13                  # padded width (H + 2*3)
PHW = PW * PW            # 169
ROWSZ = B * HW           # 196 payload per dumped row
RSTRIDE = 200            # dumped row stride (gather walks at 201)
F32 = mybir.dt.float32
BF16 = mybir.dt.bfloat16

_cached = {}
last_exec_time_ns = None


def build():
    nc = bacc.Bacc(
        "TRN2", target_bir_lowering=False, debug=False, num_devices=NCORES
    )
    x_d = nc.dram_tensor("x", (B, C, H, W), F32, kind="ExternalInput")
    wk_d = nc.dram_tensor("Wk", (P,), F32, kind="ExternalInput")
    bk_d = nc.dram_tensor("bk", (P,), F32, kind="ExternalInput")
    ba_d = nc.dram_tensor("b_adap", (P,), F32, kind="ExternalInput")
    out_d = nc.dram_tensor("out", (B, P, H, W), F32, kind="ExternalOutput")
    stagE_d = nc.dram_tensor(
        "stagE_scratch", (HW * RSTRIDE + 400,), F32, kind="Internal"
    )
    stagF_d = nc.dram_tensor(
        "stagF_scratch", (HW * RSTRIDE + 400,), F32, kind="Internal"
    )

    with tile.TileContext(nc) as tc:
        with (
            tc.tile_pool(name="sb", bufs=1) as sb,
            tc.tile_pool(name="ps", bufs=1, space="PSUM") as ps,
        ):
            xsb = sb.tile([128, 2, B, HW], F32)       # x as loaded
            # bf16 x columns 0:49, ones columns 49:98 (Gram + sum fused)
            xbf = sb.tile([128, 2, B, 2 * HW], BF16)
            sel = sb.tile([2 * HW, 2], F32)           # block-row selector
            params = sb.tile([2, P], F32)             # Wk; bk
            pbf = sb.tile([2, P], BF16)
            badap = sb.tile([128, 2], F32)            # b_adap, chunked
            gxs = sb.tile([2 * HW, B, HW], F32)       # Gram/chan-sum rows
            mask = sb.tile([2 * HW, B, 56], mybir.dt.uint32)  # band validity
            EFW = sb.tile([2 * HW, B, 8, 7], F32)     # gathered wide bands
            EFM = sb.tile([2 * HW, B, 8, 7], F32)     # masked bands (padded)
            R = sb.tile([2, B * HW], BF16)            # S1; S2
            ysb = sb.tile([128, 2, B, HW], F32)

            GX_ps = [ps.tile([2 * HW, 2, HW], F32, name=f"gx{h}") for h in range(2)]
            S_ps = ps.tile([2, B * HW], F32)
            y0_ps = ps.tile([128, B * HW], F32)
            y1_ps = ps.tile([128, B * HW], F32)

            sel_np = np.zeros((2 * HW, 2), dtype=np.float32)
            sel_np[0:HW, 0] = 1.0
            sel_np[HW : 2 * HW, 1] = 1.0
            sel_d = nc.inline_tensor(sel_np, name="sel_const")

            # mask[(a,b), n, i, j] = [0 <= a+i-3 < 7] * [0 <= b+j-3 < 7]
            aa, bb, ii, jj = np.meshgrid(
                np.arange(7), np.arange(7), np.arange(7), np.arange(7),
                indexing="ij",
            )
            m49 = (
                ((aa + ii - 3) >= 0) & ((aa + ii - 3) < 7)
                & ((bb + jj - 3) >= 0) & ((bb + jj - 3) < 7)
            ).astype(np.uint32).reshape(HW, 1, 49)
            mask_np = np.zeros((2 * HW, B, 56), dtype=np.uint32)
            mask_np[0:HW, :, 0:49] = m49
            mask_np[HW : 2 * HW, :, 0:49] = m49
            mask_d = nc.inline_tensor(mask_np, name="mask_const")

            nc.vector.memset(xbf[:, :, :, HW : 2 * HW], 1.0)
            nc.vector.memset(EFM[:], 0.0)
            nc.gpsimd.dma_start(sel[:], sel_d[:])
            nc.gpsimd.dma_start(mask[:], mask_d[:])

            # x -> SBUF with channels on partitions (two 128-chunks),
            # split 4 ways for DMA queue parallelism
            # pair the two channel chunks of each sample half on different
            # DGE queues so the first Gram accumulation pair is unblocked
            # as early as possible
            xr = x_d.ap().rearrange("n (k c) h w -> k c n (h w)", k=2)
            nc.sync.dma_start(xsb[:, 0, 0:2], xr[0, :, 0:2])
            nc.scalar.dma_start(xsb[:, 1, 0:2], xr[1, :, 0:2])
            nc.sync.dma_start(xsb[:, 0, 2:4], xr[0, :, 2:4])
            nc.scalar.dma_start(xsb[:, 1, 2:4], xr[1, :, 2:4])
            # f32 -> bf16 for the TensorEngine, pipelined with arrival
            for ck in range(2):
                for nh in range(2):
                    nc.vector.tensor_copy(
                        xbf[:, ck, 2 * nh : 2 * nh + 2, 0:HW],
                        xsb[:, ck, 2 * nh : 2 * nh + 2],
                    )

            nc.gpsimd.dma_start(params[0:1, :], wk_d.ap().unsqueeze(0))
            nc.gpsimd.dma_start(params[1:2, :], bk_d.ap().unsqueeze(0))
            # b_adap -> (128, 2): partition p, chunk k holds b_adap[k*128+p]
            nc.gpsimd.dma_start(badap[:], AP(ba_d, 0, [[1, 128], [128, 2]]))
            nc.gpsimd.tensor_copy(pbf[:], params[:])

            # fused Gram + replicated channel-sum rows, contract channels
            for b in range(B):
                for ck in range(2):
                    nc.tensor.matmul(
                        GX_ps[b // 2][:, b % 2, :],
                        xbf[:, ck, b, :],
                        xbf[:, ck, b, 0:HW],
                        start=(ck == 0),
                        stop=(ck == 1),
                    )

            # two pipelined dump->gather chains (E rows on the Sync DGE,
            # F rows on the Activation DGE): row r dumped at 24 + 200*r;
            # the band for (r, n) starts at within-row offset r - 24, so
            # the gather walks rows at 201.
            for nh in range(2):
                ns = slice(2 * nh, 2 * nh + 2)
                nc.vector.tensor_copy(gxs[:, ns], GX_ps[nh][:])
            stag_pat = [[RSTRIDE, HW], [1, ROWSZ]]
            gat_pat = [[RSTRIDE + 1, HW], [HW, B], [1, 55]]
            efw_v = EFW[:].rearrange("r b i j -> r b (i j)")
            nc.sync.dma_start(AP(stagE_d, 24, stag_pat), gxs[0:HW])
            nc.scalar.dma_start(AP(stagF_d, 24, stag_pat), gxs[HW : 2 * HW])
            nc.sync.dma_start(
                efw_v[0:HW, :, 0:55], AP(stagE_d, 0, gat_pat)
            )
            nc.scalar.dma_start(
                efw_v[HW : 2 * HW, :, 0:55], AP(stagF_d, 0, gat_pat)
            )

            # zero the wrapped reads, then block-row reduce: S_ps = [S1; S2]
            nc.vector.copy_predicated(
                EFM[:].rearrange("r b i j -> r b (i j)")[:, :, 0:49],
                mask[:, :, 0:49],
                EFW[:, :, 0:7, :].rearrange("r b i j -> r b (i j)"),
            )
            nc.tensor.matmul(
                S_ps[:],
                sel[:],
                EFM[:].rearrange("r b i j -> r b (i j)")[:, :, 0:49],
                start=True,
                stop=True,
            )
            nc.vector.tensor_copy(R[:], S_ps[:])

            # y[p, n, i, j] = Wk[p]*S1 + bk[p]*S2   (+ b_adap via bias)
            outr = out_d.ap().rearrange("n (k p) h w -> k p n (h w)", k=2)
            for pk, yps in enumerate([y0_ps, y1_ps]):
                nc.tensor.matmul(
                    yps[:],
                    pbf[:, pk * 128 : (pk + 1) * 128],
                    R[:],
                    start=True,
                    stop=True,
                )
                if pk == 0:
                    nc.scalar.activation(
                        ysb[:, pk],
                        yps[:].rearrange("p (b s) -> p b s", b=B),
                        mybir.ActivationFunctionType.Identity,
                        bias=badap[:, pk : pk + 1],
                    )
                else:
                    nc.vector.tensor_scalar_add(
                        ysb[:, pk],
                        yps[:].rearrange("p (b s) -> p b s", b=B),
                        badap[:, pk : pk + 1],
                    )
                (nc.sync if pk == 0 else nc.scalar).dma_start(outr[pk], ysb[:, pk])

    nc.compile()
    return nc


def kernel(x, Wk, bk, b_adap):
    global last_exec_time_ns
    if "nc" not in _cached:
        _cached["nc"] = build()
    nc = _cached["nc"]

    x = np.ascontiguousarray(x, dtype=np.float32)
    Wk = np.ascontiguousarray(Wk, dtype=np.float32)
    bk = np.ascontiguousarray(bk, dtype=np.float32)
    b_adap = np.ascontiguousarray(b_adap, dtype=np.float32)

    in_maps = [
        {"x": x[i * B : (i + 1) * B], "Wk": Wk, "bk": bk, "b_adap": b_adap}
        for i in range(NCORES)
    ]
    res = run_bass_kernel_spmd(
        nc,
        in_maps,
        core_ids=list(range(NCORES)),
        trace=bool(os.environ.get("KERNEL_TRACE")),
    )
    last_exec_time_ns = res.exec_time_ns
    out = np.concatenate(
        [res.results[i]["out"].reshape(B, P, H, W) for i in range(NCORES)], axis=0
    )
    return out


# revision 22
# speedup vs baseline: 1.1265x; 1.1265x over previous
"""AdaptiveKernelFC Trainium2 kernel (8-core data parallel).

Math: the reference builds per-sample filters w[n,p,c,kh,kw] =
x[n,c,kh,kw]*Wk[p] + bk[p] and convolves x[n] with them (7x7 kernel ==
feature map size, pad 3).  The conv factors exactly:

    y[n,p,i,j] = Wk[p]*S1[n,i,j] + bk[p]*S2[n,i,j] + b_adap[p]

with
    S1[n,i,j] = sum_{a,b} G[n,(a,b),(a+i-3,b+j-3)]      (Gram diag bands)
    G[n,r,q]  = sum_c x[n,c,r] * x[n,c,q]               (49x49 per sample)
    S2[n,i,j] = sum_{a,b} xspad[n,i+a,j+b],  xs = sum_c x[n,c]

Pipeline per core (4 samples):
  1. One fused matmul pair per sample: lhsT = [x_chunk | ones49] so PSUM
     rows 0-48 are the Gram matrix and rows 49-97 are 49 replicated
     copies of the channel sums.
  2. DVE-copy into a zero-padded 13x13 layout (98, B, 13, 13).
  3. Dump to DRAM with a per-row stagger: row r=(a,b) lands at
     683*r - 13a - b.  This makes the diagonal-band gather uniform:
     element (r, n, i*13+j') sits at 683*r + 169*n + (i*13+j'), so ONE
     3-dim DMA gathers every 91-wide band for all 98 rows x 4 samples.
  4. One selector matmul (98,2)^T @ (98, B,7,7 view) -> [S1; S2].
  5. K=2 matmul against [Wk; bk], then ScalarE Identity-with-bias adds
     b_adap while moving PSUM -> SBUF.

Sharding: pure data parallel, batch N=32 split 4 samples/core across 8
cores; params replicated; outputs concatenated.
"""

import os
import numpy as np

import concourse.bass as bass
import concourse.bacc as bacc
import concourse.mybir as mybir
import concourse.tile as tile
import ml_dtypes

jnp_bf16 = ml_dtypes.bfloat16
from concourse.ap import AP
from concourse.bass_utils import run_bass_kernel_spmd

N, C, H, W = 32, 256, 7, 7
P = 256
NCORES = 8
B = N // NCORES          # samples per core
HW = H * W               # 49
PW = 13                  # padded width (H + 2*3)
PHW = PW * PW            # 169
ROWSZ = B * HW           # 196 payload per dumped row
RSTRIDE = 200            # dumped row stride (gather walks at 201)
F32 = mybir.dt.float32
BF16 = mybir.dt.bfloat16

_cached = {}
last_exec_time_ns = None


def build():
    nc = bacc.Bacc(
        "TRN2", target_bir_lowering=False, debug=False, num_devices=NCORES
    )
    x_d = nc.dram_tensor("x", (B, C, H, W), F32, kind="ExternalInput")
    wk_d = nc.dram_tensor("Wk", (P,), F32, kind="ExternalInput")
    bk_d = nc.dram_tensor("bk", (P,), F32, kind="ExternalInput")
    ba_d = nc.dram_tensor("b_adap", (P,), F32, kind="ExternalInput")
    out_d = nc.dram_tensor("out", (B, P, H, W), F32, kind="ExternalOutput")
    stag_d = [
        [
            nc.dram_tensor(
                f"stag{blk}{nh}_scratch", (HW * 100 + 200,), F32, kind="Internal"
            )
            for nh in range(2)
        ]
        for blk in range(2)
    ]

    with tile.TileContext(nc) as tc:
        with (
            tc.tile_pool(name="sb", bufs=1) as sb,
            tc.tile_pool(name="ps", bufs=1, space="PSUM") as ps,
        ):
            xsb = sb.tile([128, 2, B, HW], F32)       # x as loaded
            # bf16 x columns 0:49, ones columns 49:98 (Gram + sum fused)
            xbf = sb.tile([128, 2, B, 2 * HW], BF16)
            sel = sb.tile([2 * HW, 2], F32)           # block-row selector
            params = sb.tile([2, P], F32)             # Wk; bk
            pbf = sb.tile([2, P], BF16)
            badap = sb.tile([128, 2], F32)            # b_adap, chunked
            gxs = sb.tile([2 * HW, B, HW], F32)       # Gram/chan-sum rows
            mask = sb.tile([2 * HW, B, 56], mybir.dt.uint32)  # band validity
            EFW = sb.tile([2 * HW, B, 8, 7], F32)     # gathered wide bands
            EFM = sb.tile([2 * HW, B, 8, 7], F32)     # masked bands (padded)
            R = sb.tile([2, B * HW], BF16)            # S1; S2
            ysb = sb.tile([128, 2, B, HW], F32)

            GX_ps = [ps.tile([2 * HW, 2, HW], F32, name=f"gx{h}") for h in range(2)]
            S_ps = ps.tile([2, B * HW], F32)
            y0_ps = ps.tile([128, B * HW], F32)
            y1_ps = ps.tile([128, B * HW], F32)

            sel_np = np.zeros((2 * HW, 2), dtype=np.float32)
            sel_np[0:HW, 0] = 1.0
            sel_np[HW : 2 * HW, 1] = 1.0
            sel_d = nc.inline_tensor(sel_np, name="sel_const")

            # mask[(a,b), n, i, j] = [0 <= a+i-3 < 7] * [0 <= b+j-3 < 7]
            aa, bb, ii, jj = np.meshgrid(
                np.arange(7), np.arange(7), np.arange(7), np.arange(7),
                indexing="ij",
            )
            m49 = (
                ((aa + ii - 3) >= 0) & ((aa + ii - 3) < 7)
                & ((bb + jj - 3) >= 0) & ((bb + jj - 3) < 7)
            ).astype(np.uint32).reshape(HW, 1, 49)
            mask_np = np.zeros((2 * HW, B, 56), dtype=np.uint32)
            mask_np[0:HW, :, 0:49] = m49
            mask_np[HW : 2 * HW, :, 0:49] = m49
            mask_d = nc.inline_tensor(mask_np, name="mask_const")

            nc.vector.memset(xbf[:, :, :, HW : 2 * HW], 1.0)
            nc.vector.memset(EFM[:], 0.0)
            nc.gpsimd.dma_start(sel[:], sel_d[:])
            nc.gpsimd.dma_start(mask[:], mask_d[:])

            # x -> SBUF with channels on partitions (two 128-chunks),
            # split 4 ways for DMA queue parallelism
            # pair the two channel chunks of each sample half on different
            # DGE queues so the first Gram accumulation pair is unblocked
            # as early as possible
            xr = x_d.ap().rearrange("n (k c) h w -> k c n (h w)", k=2)
            nc.sync.dma_start(xsb[:, 0, 0:2], xr[0, :, 0:2])
            nc.scalar.dma_start(xsb[:, 1, 0:2], xr[1, :, 0:2])
            nc.sync.dma_start(xsb[:, 0, 2:4], xr[0, :, 2:4])
            nc.scalar.dma_start(xsb[:, 1, 2:4], xr[1, :, 2:4])
            # f32 -> bf16 for the TensorEngine, pipelined with arrival
            for ck in range(2):
                for nh in range(2):
                    nc.vector.tensor_copy(
                        xbf[:, ck, 2 * nh : 2 * nh + 2, 0:HW],
                        xsb[:, ck, 2 * nh : 2 * nh + 2],
                    )

            nc.gpsimd.dma_start(params[0:1, :], wk_d.ap().unsqueeze(0))
            nc.gpsimd.dma_start(params[1:2, :], bk_d.ap().unsqueeze(0))
            # b_adap -> (128, 2): partition p, chunk k holds b_adap[k*128+p]
            nc.gpsimd.dma_start(badap[:], AP(ba_d, 0, [[1, 128], [128, 2]]))
            nc.gpsimd.tensor_copy(pbf[:], params[:])

            # fused Gram + replicated channel-sum rows, contract channels
            for b in range(B):
                for ck in range(2):
                    nc.tensor.matmul(
                        GX_ps[b // 2][:, b % 2, :],
                        xbf[:, ck, b, :],
                        xbf[:, ck, b, 0:HW],
                        start=(ck == 0),
                        stop=(ck == 1),
                    )

            # 4 pipelined dump->gather chains (E/F rows x sample halves):
            # row r dumped at 24 + 100*r with the half's 98-elem payload;
            # the band for (r, n) starts at within-row offset r - 24, so
            # the gather walks rows at 101.  E chains on the Sync DGE, F
            # chains on the Activation DGE; the first half's round trip
            # overlaps the second half's Gram matmuls.
            stag_pat = [[100, HW], [1, 2 * HW]]
            gat_pat = [[101, HW], [HW, 2], [1, 55]]
            efw_v = EFW[:].rearrange("r b i j -> r b (i j)")
            for nh in range(2):
                ns = slice(2 * nh, 2 * nh + 2)
                nc.vector.tensor_copy(gxs[:, ns], GX_ps[nh][:])
                nc.sync.dma_start(
                    AP(stag_d[0][nh], 24, stag_pat), gxs[0:HW, ns]
                )
                nc.scalar.dma_start(
                    AP(stag_d[1][nh], 24, stag_pat), gxs[HW : 2 * HW, ns]
                )
            for nh in range(2):
                ns = slice(2 * nh, 2 * nh + 2)
                nc.sync.dma_start(
                    efw_v[0:HW, ns, 0:55], AP(stag_d[0][nh], 0, gat_pat)
                )
                nc.scalar.dma_start(
                    efw_v[HW : 2 * HW, ns, 0:55], AP(stag_d[1][nh], 0, gat_pat)
                )

            # per-half output pipeline: mask -> selector matmul -> R cast
            # -> final matmuls -> bias add -> output DMA, so half 0's
            # output path overlaps half 1's gather latency
            efm_v = EFM[:].rearrange("r b i j -> r b (i j)")
            efw7_v = EFW[:, :, 0:7, :].rearrange("r b i j -> r b (i j)")
            outr = out_d.ap().rearrange("n (k p) h w -> k p n (h w)", k=2)
            yv = [
                yps[:].rearrange("p (b s) -> p b s", b=B)
                for yps in (y0_ps, y1_ps)
            ]
            for nh in range(2):
                ns = slice(2 * nh, 2 * nh + 2)
                nc.vector.copy_predicated(
                    efm_v[:, ns, 0:49], mask[:, ns, 0:49], efw7_v[:, ns]
                )
                nc.tensor.matmul(
                    S_ps[:, 2 * nh * HW : (2 * nh + 2) * HW],
                    sel[:],
                    efm_v[:, ns, 0:49],
                    start=True,
                    stop=True,
                )
                nc.vector.tensor_copy(
                    R[:, 2 * nh * HW : (2 * nh + 2) * HW],
                    S_ps[:, 2 * nh * HW : (2 * nh + 2) * HW],
                )
                for pk, yps in enumerate([y0_ps, y1_ps]):
                    nc.tensor.matmul(
                        yv[pk][:, ns],
                        pbf[:, pk * 128 : (pk + 1) * 128],
                        R[:].rearrange("t (b s) -> t b s", b=B)[:, ns],
                        start=True,
                        stop=True,
                    )
                    if pk == 0:
                        nc.scalar.activation(
                            ysb[:, pk, ns],
                            yv[pk][:, ns],
                            mybir.ActivationFunctionType.Identity,
                            bias=badap[:, pk : pk + 1],
                        )
                    else:
                        nc.vector.tensor_scalar_add(
                            ysb[:, pk, ns],
                            yv[pk][:, ns],
                            badap[:, pk : pk + 1],
                        )
                    (nc.sync if pk == 0 else nc.scalar).dma_start(
                        outr[pk][:, ns], ysb[:, pk, ns]
                    )

    nc.compile()
    return nc


def kernel(x, Wk, bk, b_adap):
    global last_exec_time_ns
    if "nc" not in _cached:
        _cached["nc"] = build()
    nc = _cached["nc"]

    x = np.ascontiguousarray(x, dtype=np.float32)
    Wk = np.ascontiguousarray(Wk, dtype=np.float32)
    bk = np.ascontiguousarray(bk, dtype=np.float32)
    b_adap = np.ascontiguousarray(b_adap, dtype=np.float32)

    in_maps = [
        {"x": x[i * B : (i + 1) * B], "Wk": Wk, "bk": bk, "b_adap": b_adap}
        for i in range(NCORES)
    ]
    res = run_bass_kernel_spmd(
        nc,
        in_maps,
        core_ids=list(range(NCORES)),
        trace=bool(os.environ.get("KERNEL_TRACE")),
    )
    last_exec_time_ns = res.exec_time_ns
    out = np.concatenate(
        [res.results[i]["out"].reshape(B, P, H, W) for i in range(NCORES)], axis=0
    )
    return out
